# revision 4
# baseline (speedup 1.0000x reference)
"""BEVDet lift-splat kernel for 8 Trainium2 NeuronCores.

Strategy (per spec sharding_hint: "shard the BEV grid spatially ... and route
points by coor"): the BEV grid is sharded over 8 cores (8192 cells each).
During input sharding, points are routed by `lidar_coor_1d`: the last-write-wins
resolution (pure index formatting: winner[coor] = point_id, one vectorized
assignment) gives each grid cell its source point. Each core then computes the
depth_net (1x1 conv as matmuls) + softmax over depth bins for the full image
table, gathers its cells' (tran|depth) feature rows with hardware dma_gather,
multiplies, applies the bev_feat fallback for empty cells, transposes to the
output layout on the TensorEngine, and writes its [64, 8192] output slice.
"""
import sys
sys.path.insert(0, "/opt/trn_rl_repo")
import numpy as np
import concourse.bass as bass
import concourse.bacc as bacc
import concourse.tile as tile
import concourse.mybir as mybir
from concourse.bass_utils import run_bass_kernel_spmd

N_CAM, CIN, H, W = 6, 256, 32, 88
HW = H * W                     # 2816
NHW = N_CAM * HW               # 16896
DD, C = 59, 64                 # depth bins, channels
NPTS = N_CAM * DD * HW         # 996864
G = 65536
SENT = G
NCORES = 8
CPC = G // NCORES              # 8192 cells per core
TILES_PER_CAM = HW // 128      # 22
F32 = mybir.dt.float32

_cache = {}


def _build():
    nc = bacc.Bacc("TRN2", target_bir_lowering=True, debug=False)
    xs = nc.dram_tensor("xs", [N_CAM, 2, 128, HW], F32, kind="ExternalInput")
    wT = nc.dram_tensor("wT", [2, 128, 123], F32, kind="ExternalInput")
    brow = nc.dram_tensor("brow", [1, 123], F32, kind="ExternalInput")
    ones_r = nc.dram_tensor("ones_r", [1, 128], F32, kind="ExternalInput")
    ident = nc.dram_tensor("ident", [128, 128], F32, kind="ExternalInput")
    colw16 = nc.dram_tensor("colw16", [128, CPC // 16], mybir.dt.int16, kind="ExternalInput")
    onehot = nc.dram_tensor("onehot", [128, (CPC // 128) * C], F32, kind="ExternalInput")
    validm = nc.dram_tensor("validm", [128, CPC // 128], F32, kind="ExternalInput")
    bevs = nc.dram_tensor("bevs", [128, (CPC // 128) * C], F32, kind="ExternalInput")
    ft = nc.dram_tensor("ft", [NHW, 128], F32)
    out_sl = nc.dram_tensor("out_sl", [C, CPC], F32, kind="ExternalOutput")

    with tile.TileContext(nc) as tc:
        with (
            tc.tile_pool(name="xpool", bufs=2) as xpool,
            tc.tile_pool(name="wpool", bufs=1) as wpool,
            tc.tile_pool(name="cpool", bufs=4) as cpool,
            tc.tile_pool(name="spool", bufs=4) as spool,
            tc.tile_pool(name="psum", bufs=4, space="PSUM") as pp,
            tc.tile_pool(name="gpool", bufs=1) as gpool,
            tc.tile_pool(name="psum2", bufs=4, space="PSUM") as pp2,
        ):
            w_sb0 = wpool.tile([128, 123], F32)
            w_sb1 = wpool.tile([128, 123], F32)
            b_sb = wpool.tile([1, 123], F32)
            o_sb = wpool.tile([1, 128], F32)
            id_sb = wpool.tile([128, 128], F32)
            nc.sync.dma_start(out=w_sb0[:], in_=wT[0])
            nc.sync.dma_start(out=w_sb1[:], in_=wT[1])
            nc.sync.dma_start(out=b_sb[:], in_=brow[:])
            nc.sync.dma_start(out=o_sb[:], in_=ones_r[:])
            nc.sync.dma_start(out=id_sb[:], in_=ident[:])

            # ---- Phase B: depth_net + softmax -> ft[NHW, 128] rows [tran64|depth59|pad5]
            for cam in range(N_CAM):
                x_sb0 = xpool.tile([128, HW], F32)
                x_sb1 = xpool.tile([128, HW], F32)
                nc.sync.dma_start(out=x_sb0[:], in_=xs[cam, 0])
                nc.sync.dma_start(out=x_sb1[:], in_=xs[cam, 1])
                for t in range(TILES_PER_CAM):
                    cs = t * 128
                    ps = pp.tile([128, 123], F32, space="PSUM")
                    nc.tensor.matmul(ps[:], lhsT=x_sb0[:, cs:cs + 128],
                                     rhs=w_sb0[:], start=True, stop=False)
                    nc.tensor.matmul(ps[:], lhsT=x_sb1[:, cs:cs + 128],
                                     rhs=w_sb1[:], start=False, stop=False)
                    nc.tensor.matmul(ps[:], lhsT=o_sb[:], rhs=b_sb[:],
                                     start=False, stop=True)
                    comb = cpool.tile([128, 128], F32)
                    mx = spool.tile([128, 1], F32)
                    nmx = spool.tile([128, 1], F32)
                    ssum = spool.tile([128, 1], F32)
                    rs = spool.tile([128, 1], F32)
                    nc.vector.tensor_reduce(out=mx[:], in_=ps[:, 0:DD],
                                            axis=mybir.AxisListType.X,
                                            op=mybir.AluOpType.max)
                    nc.vector.tensor_scalar_mul(nmx[:], mx[:], -1.0)
                    nc.scalar.activation(comb[:, 64:64 + DD], ps[:, 0:DD],
                                         mybir.ActivationFunctionType.Exp,
                                         bias=nmx[:, :], scale=1.0,
                                         accum_out=ssum[:])
                    nc.vector.reciprocal(rs[:], ssum[:])
                    nc.vector.tensor_scalar_mul(comb[:, 64:64 + DD],
                                                comb[:, 64:64 + DD], rs[:, :])
                    nc.vector.tensor_copy(out=comb[:, 0:64], in_=ps[:, DD:123])
                    nc.vector.memset(comb[:, 123:128], 0.0)
                    nc.sync.dma_start(out=ft[cam * HW + cs:cam * HW + cs + 128, :],
                                      in_=comb[:])

            # ---- Phase C: gather this core's 8192 cells, multiply, mask, transpose
            ci_sb = gpool.tile([128, CPC // 16], mybir.dt.int16)
            oh_sb = gpool.tile([128, (CPC // 128) * C], F32)
            vm_sb = gpool.tile([128, CPC // 128], F32)
            bv_sb = gpool.tile([128, (CPC // 128) * C], F32)
            gat = gpool.tile([128, (CPC // 128) * 128], F32)
            nc.sync.dma_start(out=ci_sb[:], in_=colw16[:])
            nc.sync.dma_start(out=oh_sb[:], in_=onehot[:])
            nc.sync.dma_start(out=vm_sb[:], in_=validm[:])
            nc.sync.dma_start(out=bv_sb[:], in_=bevs[:])
            GCH = 512   # idxs per gather call (per-inst descriptor cap safety)
            for hh in range(CPC // GCH):
                nc.gpsimd.dma_gather(
                    out_ap=gat[:].rearrange("p (n d) -> p n d", d=128)[:, hh * (GCH // 128):(hh + 1) * (GCH // 128), :],
                    in_ap=ft[:, :],
                    idxs_ap=ci_sb[:, hh * (GCH // 16):(hh + 1) * (GCH // 16)],
                    num_idxs=GCH, num_idxs_reg=GCH, elem_size=128)
            g3 = gat[:].rearrange("p (n d) -> p n d", d=128)
            oh3 = oh_sb[:].rearrange("p (n d) -> p n d", d=C)
            bv3 = bv_sb[:].rearrange("p (n d) -> p n d", d=C)
            prod = gpool.tile([128, (CPC // 128) * C], F32)
            p3 = prod[:].rearrange("p (n d) -> p n d", d=C)
            nc.vector.tensor_tensor(out=p3, in0=g3[:, :, 64:128], in1=oh3,
                                    op=mybir.AluOpType.mult)
            dsel = gpool.tile([128, CPC // 128], F32)
            nc.vector.tensor_reduce(out=dsel[:].rearrange("p (n d) -> p n d", d=1),
                                    in_=p3, axis=mybir.AxisListType.X,
                                    op=mybir.AluOpType.add)
            outf = gpool.tile([128, (CPC // 128) * C], F32)
            of3 = outf[:].rearrange("p (n d) -> p n d", d=C)
            d3 = dsel[:].rearrange("p (n d) -> p n d", d=1).to_broadcast([128, CPC // 128, C])
            nc.vector.tensor_tensor(out=of3, in0=g3[:, :, 0:64], in1=d3,
                                    op=mybir.AluOpType.mult)
            # valid? outf : bev
            v3 = vm_sb[:].rearrange("p (n d) -> p n d", d=1).to_broadcast([128, CPC // 128, C])
            nc.vector.tensor_tensor(out=of3, in0=of3, in1=v3, op=mybir.AluOpType.mult)
            ivm = gpool.tile([128, CPC // 128], F32)
            nc.vector.tensor_scalar(out=ivm[:], in0=vm_sb[:], scalar1=-1.0,
                                    scalar2=1.0, op0=mybir.AluOpType.mult,
                                    op1=mybir.AluOpType.add)
            iv3 = ivm[:].rearrange("p (n d) -> p n d", d=1).to_broadcast([128, CPC // 128, C])
            tmpb = gpool.tile([128, (CPC // 128) * C], F32)
            tb3 = tmpb[:].rearrange("p (n d) -> p n d", d=C)
            nc.vector.tensor_tensor(out=tb3, in0=bv3, in1=iv3, op=mybir.AluOpType.mult)
            nc.vector.tensor_tensor(out=of3, in0=of3, in1=tb3, op=mybir.AluOpType.add)
            # transpose [128 cells, 64] tiles -> [64, 128] and emit
            osb = gpool.tile([C, CPC], F32)
            for ch in range(CPC // 128):
                pt = pp2.tile([C, 128], F32, space="PSUM")
                nc.tensor.transpose(out=pt[:], in_=of3[:, ch, :], identity=id_sb[:])
                nc.vector.tensor_copy(out=osb[:, ch * 128:(ch + 1) * 128], in_=pt[:])
            nc.sync.dma_start(out=out_sl[:, :], in_=osb[:])
    nc.compile()
    return nc


def _wrap16(f, cw):
    n = f.shape[0]
    a = np.zeros((16, cw), f.dtype)
    a[np.arange(n) % 16, np.arange(n) // 16] = f
    return np.tile(a, (8, 1))


def _w128(v):
    """[8192]-per-cell -> [128, 64] layout (cell j -> (j%128, j//128))."""
    n = v.shape[0]
    a = np.zeros((128, n // 128) + v.shape[1:], v.dtype)
    a[np.arange(n) % 128, np.arange(n) // 128] = v
    return a.reshape(128, -1)


def kernel(**inputs):
    x_in = np.ascontiguousarray(np.asarray(inputs["x_in"], np.float32))
    W_dn = np.asarray(inputs["W_dn"], np.float32)
    b_dn = np.asarray(inputs["b_dn"], np.float32)
    coor = np.asarray(inputs["lidar_coor_1d"]).astype(np.int64)
    bev_feat = np.asarray(inputs["bev_feat"], np.float32)

    # ---- route points by coor (sharding prep): last-write-wins winner ids
    winner = np.zeros(G + 1, np.int64)
    keep = coor != SENT
    ids = np.arange(NPTS, dtype=np.int64)
    winner[coor[keep]] = ids[keep] + 1
    w1 = winner[:G]                      # id+1 per cell, 0 = none
    valid = w1 > 0
    pm = np.maximum(w1 - 1, 0)
    t = pm // HW
    hwi = pm % HW
    n_i = t // DD
    d_i = t % DD
    col = (n_i * HW + hwi).astype(np.int32)

    xs = x_in.transpose(0, 1, 2, 3).reshape(N_CAM, 2, 128, HW)
    wT = W_dn.T.reshape(2, 128, 123).astype(np.float32)
    brow = b_dn.reshape(1, 123)
    ones_r = np.ones((1, 128), np.float32)
    ident = np.eye(128, dtype=np.float32)

    if "nc" not in _cache:
        _cache["nc"] = _build()
    nc = _cache["nc"]

    in_maps = []
    for k in range(NCORES):
        sl = slice(k * CPC, (k + 1) * CPC)
        colk = col[sl]
        dk = d_i[sl]
        vk = valid[sl].astype(np.float32)
        oh = np.zeros((CPC, C), np.float32)
        oh[np.arange(CPC), np.minimum(dk, C - 1)] = vk    # selects depth slot 64+d
        in_maps.append({
            "xs": xs, "wT": wT, "brow": brow, "ones_r": ones_r, "ident": ident,
            "colw16": _wrap16(colk.astype(np.int16), CPC // 16),
            "onehot": _w128(oh),
            "validm": _w128(vk),
            "bevs": _w128(bev_feat[sl].astype(np.float32)),
        })

    res = run_bass_kernel_spmd(nc, in_maps, core_ids=list(range(NCORES)))
    out = np.empty((C, G), np.float32)
    for k in range(NCORES):
        out[:, k * CPC:(k + 1) * CPC] = res.results[k]["out_sl"]
    return out.reshape(1, C, 256, 256)


if __name__ == "__main__":
    pass
